# revision 25
# baseline (speedup 1.0000x reference)
"""Self-attention (Base_OC / SAGAN-style) module on Trainium2, 8 NeuronCores.

Problem: x[4, 64, 64, 512]; per batch element b (N = 4096 tokens, C = 512):
  f = x@wf+bf [N,64]; g = x@wg+bg [N,64]; hv = x@wh+bh [N,256]
  s = g @ f^T [N,N]; beta = softmax(s); o = beta @ hv [N,256]
  att = gamma*(o@wo+bo) + x; y = relu(BN([att,x] @ wc + bc))

Sharding: 8 cores = batch(4) x query-row-halves(2). Each core receives x[b]
permuted so its own 2048 query rows come first (attention is permutation-
invariant over keys), computes the pipeline for those rows, returns [2048,512].

Layout: x is PE-transposed once to xT [c, n]. All dense matmuls run in bf16
(fp32[r] matmuls execute in fp32-HIGH mode at ~1/4 rate on TRN2 HW; bf16
streams 1 col/cycle). Accumulation stays fp32 in PSUM. Softmax skips the
max-subtraction (max |logit| ~ 67, exp stays in fp32/bf16 range); the softmax
denominator comes from a ones-column appended to hv. s-stage matmuls (K=64)
run pairwise-concurrent in the PE array via tile_position row packing, with
f/g duplicated to both partition halves by SBUF->SBUF DMA. exp runs in
[128,1024] chunks (2 key tiles) to amortize the ~293ns ACT fixed cost.

Host-side algebra fold: y = relu(o@W1 + x@Wx + brow) with
  W1 = gamma*(wo @ wcx1), Wx = wcx1 + wcx2, brow = gamma*bo@wcx1 + bn-fold,
where wcx = wc * bn_scale/sqrt(var+eps). This removes the o@wo stage and the
att-half of the concat matmul entirely (no gamma-residual vector work).
"""

import numpy as np
import ml_dtypes

import concourse.bacc as bacc
import concourse.mybir as mybir
import concourse.tile as tile
from concourse.bass_utils import run_bass_kernel_spmd

FP = mybir.dt.float32
BF = mybir.dt.bfloat16
AF = mybir.ActivationFunctionType
OP = mybir.AluOpType
BF_NP = ml_dtypes.bfloat16

N_FULL, N_OWN, C, D8, D2 = 4096, 2048, 512, 64, 256
NMT = N_FULL // 128   # 32 key tiles
NCT = C // 128        # 4 channel tiles
NET = D2 // 128       # 2 e tiles
NNB = N_OWN // 512    # 4 query blocks per core
HW2 = 258             # hv width: 256 values | ones col | pad
EPS = 1e-3


def build_program(reps=1):
    nc = bacc.Bacc("TRN2", target_bir_lowering=False, debug=False, num_devices=8)

    xt_d = nc.dram_tensor("xt", [C, N_FULL], BF, kind="ExternalInput").ap()
    wfg_d = nc.dram_tensor("wfg", [C, 128], BF, kind="ExternalInput").ap()
    bfg_d = nc.dram_tensor("bfg", [128, 1], FP, kind="ExternalInput").ap()
    whx_d = nc.dram_tensor("whx", [C, HW2], BF, kind="ExternalInput").ap()
    bhbc_d = nc.dram_tensor("bhbc", [128, HW2], FP, kind="ExternalInput").ap()
    w1x_d = nc.dram_tensor("w1x", [D2, C], BF, kind="ExternalInput").ap()
    wxx_d = nc.dram_tensor("wxx", [C, C], BF, kind="ExternalInput").ap()
    bcbc_d = nc.dram_tensor("bcbc", [128, C], FP, kind="ExternalInput").ap()
    ident_d = nc.dram_tensor("identr", [128, 128], BF, kind="ExternalInput").ap()
    y_d = nc.dram_tensor("y", [N_OWN, C], BF, kind="ExternalOutput").ap()

    with tile.TileContext(nc) as tc:
        with (
            tc.tile_pool(name="consts", bufs=1) as cpool,
            tc.tile_pool(name="big", bufs=1) as bigp,
            tc.tile_pool(name="stream", bufs=2) as sp,
            tc.tile_pool(name="exps", bufs=4) as exp_pool,
            tc.tile_pool(name="psB_s", bufs=1, space="PSUM") as ps_pool,
            tc.tile_pool(name="psB_u", bufs=1, space="PSUM") as pu,
        ):
            xT = bigp.tile([128, NCT * N_FULL], BF)   # 32 KB/part
            fT = bigp.tile([128, N_FULL], BF)         # rows 0:64 f, 64:128 ZERO
            gT = bigp.tile([128, N_OWN], BF)          # rows 0:64 g, 64:128 copy
            hv = bigp.tile([128, NMT * HW2], BF)      # 16.5 KB
            # zero fT's lower half once: s-matmuls run full-K (128) so the
            # whole tensor stream stays uniform (no row-group transitions);
            # rows 64:128 contribute fT-zero * gT-copy = 0.
            nc.gpsimd.memset(fT[D8:128, :], 0.0)
            whx_sb = cpool.tile([128, NCT * HW2], BF)
            wfg_sb = cpool.tile([128, NCT * 128], BF)
            bfg_sb = cpool.tile([128, 1], FP)
            bhbc_sb = cpool.tile([128, HW2], FP)

            xT_v = xT.rearrange("p (t n) -> p t n", t=NCT)
            xt_dv = xt_d.rearrange("(t p) n -> p t n", p=128)

            def dma_xt(half):
                # one batched DMA per 512-token chunk (all 4 channel tiles)
                nc.sync.dma_start(
                    xT_v[:, :, half * 512:(half + 1) * 512],
                    xt_dv[:, :, half * 512:(half + 1) * 512])

            # critical-path-first DMA order: the startup chunk stays granular
            # (per-ct interleaved wfg/x/whx) so the first fg accumulation can
            # start after ~2 small DMAs; later chunks use batched prefetch.
            nc.sync.dma_start(bfg_sb, bfg_d)
            for ct in range(NCT):
                nc.sync.dma_start(wfg_sb[:, ct * 128:(ct + 1) * 128],
                                  wfg_d[ct * 128:(ct + 1) * 128, :])
                nc.sync.dma_start(
                    xT[:, ct * N_FULL: ct * N_FULL + 512],
                    xt_d[ct * 128:(ct + 1) * 128, 0:512])
                nc.sync.dma_start(whx_sb[:, ct * HW2:(ct + 1) * HW2],
                                  whx_d[ct * 128:(ct + 1) * 128, :])
            nc.sync.dma_start(bhbc_sb, bhbc_d)

            def emit_hv(mt, phv):
                hp = phv.tile([128, HW2], FP, tag="hv")
                for ct in range(NCT):
                    nc.tensor.matmul(
                        hp,
                        xT[:, ct * N_FULL + mt * 128: ct * N_FULL + (mt + 1) * 128],
                        whx_sb[:, ct * HW2:(ct + 1) * HW2],
                        start=(ct == 0), stop=(ct == NCT - 1))
                # bias (+ones col) via broadcast add, casts to bf16
                nc.vector.tensor_add(hv[:, mt * HW2:(mt + 1) * HW2], hp, bhbc_sb)

            def emit_fg(ch, pfg):
                cs = slice(ch * 512, (ch + 1) * 512)
                if ch < NNB:
                    # packed [f|g]: out rows 0:64 = f, 64:128 = g
                    fgp = pfg.tile([128, 512], FP, tag="fg")
                    for ct in range(NCT):
                        nc.tensor.matmul(
                            fgp, wfg_sb[:, ct * 128:(ct + 1) * 128],
                            xT[:, ct * N_FULL + ch * 512:
                               ct * N_FULL + (ch + 1) * 512],
                            start=(ct == 0), stop=(ct == NCT - 1))
                    nc.vector.tensor_scalar_add(fT[0:D8, cs], fgp[0:D8, :],
                                                bfg_sb[0:D8, :])
                    nc.vector.tensor_scalar_add(gT[D8:128, cs], fgp[D8:128, :],
                                                bfg_sb[D8:128, :])
                    nc.sync.dma_start(gT[0:D8, cs], gT[D8:128, cs])
                else:
                    fp_ = pfg.tile([128, 512], FP, tag="fg")
                    for ct in range(NCT):
                        nc.tensor.matmul(
                            fp_[0:D8, :], wfg_sb[:, ct * 128: ct * 128 + D8],
                            xT[:, ct * N_FULL + ch * 512:
                               ct * N_FULL + (ch + 1) * 512],
                            start=(ct == 0), stop=(ct == NCT - 1))
                    nc.vector.tensor_scalar_add(fT[0:D8, cs], fp_[0:D8, :],
                                                bfg_sb[0:D8, :])

            # one-chunk software pipeline: the u-MMs consume the PREVIOUS
            # chunk's exp so the in-order tensor queue never waits on the
            # exp that was just issued.
            su_pend = [None]   # (ex tile, mt2)

            def emit_u(up, ex, mt2):
                for half in range(2):
                    mt = 2 * mt2 + half
                    for ns in range(4):
                        nc.tensor.matmul(
                            up[:, ns * 512: ns * 512 + HW2],
                            ex[:, half * 512 + ns * 128:
                               half * 512 + (ns + 1) * 128],
                            hv[:, mt * HW2:(mt + 1) * HW2],
                            start=(mt == 0), stop=(mt == NMT - 1))

            def emit_su(nb, mt2, up):
                # two full-K (zero-padded) s-matmuls in the uniform stream,
                # one [128,1024] exp over both key tiles
                nbs = slice(nb * 512, (nb + 1) * 512)
                s2 = ps_pool.tile([128, 1024], FP, tag="s")
                for half in range(2):
                    mt = 2 * mt2 + half
                    nc.tensor.matmul(
                        s2[:, half * 512:(half + 1) * 512],
                        fT[:, mt * 128:(mt + 1) * 128],
                        gT[:, nbs], start=True, stop=True)
                ex = exp_pool.tile([128, 1024], BF, tag="expS")
                nc.scalar.activation(ex, s2, AF.Exp)
                if su_pend[0] is not None:
                    emit_u(up, *su_pend[0])
                su_pend[0] = (ex, mt2)

            def flush_su(up):
                emit_u(up, *su_pend[0])
                su_pend[0] = None

            def emit_ob(up):
                # DVE-only part of the tail: normalize u -> ob (bf16).
                # Emitted BEFORE the next block's u-matmuls so the up-bank
                # WAR dependency is tracked in the right direction.
                obs = []
                for ns in range(4):
                    rcp = sp.tile([128, 1], FP, tag="rcp")
                    nc.vector.reciprocal(rcp, up[:, ns * 512 + 256: ns * 512 + 257])
                    ob = sp.tile([128, D2], BF, tag="ob")
                    nc.vector.tensor_scalar_mul(ob, up[:, ns * 512: ns * 512 + 256],
                                                rcp)
                    obs.append(ob)
                return obs

            def emit_oT(obs, oT_cell, pm):
                # PE transposes of ob into oT (tp2 scratch in the pm ring).
                oT = sp.tile([128, NET * 512], BF, tag="oT")
                oT_cell[0] = oT
                for ns in range(4):
                    for et in range(NET):
                        tp2f = pm.tile([128, 1024], BF, tag="m", name="tp2")
                        tp2 = tp2f[:, 0:128]
                        nc.tensor.transpose(
                            tp2, obs[ns][:, et * 128:(et + 1) * 128], ident)
                        nc.vector.tensor_copy(
                            oT[:, et * 512 + ns * 128: et * 512 + (ns + 1) * 128], tp2)
                return oT

            def make_y_steps(nb, oT_cell, pm):
                # y = x@Wx + o@W1 (+brow, relu), emitted as 12 small steps so
                # they can be spread between su chunks as tensor-queue filler.
                # Returns (a, b, c) step lists, 4 each; per-ns chain a->b->c,
                # and at most two yp tiles may be live (c closes a yp).
                a_steps, b_steps, c_steps = [], [], []
                for ns in range(4):
                    state = {}

                    def s_a(ns=ns, state=state):
                        yp = pm.tile([128, 512], FP, tag="m")
                        state["yp"] = yp
                        for ct in (0, 1):
                            nc.tensor.matmul(
                                yp,
                                xT[:, ct * N_FULL + nb * 512 + ns * 128:
                                   ct * N_FULL + nb * 512 + (ns + 1) * 128],
                                wxx_sb[:, ct * C:(ct + 1) * C],
                                start=(ct == 0), stop=False)

                    def s_b(ns=ns, state=state):
                        yp = state["yp"]
                        for ct in (2, 3):
                            nc.tensor.matmul(
                                yp,
                                xT[:, ct * N_FULL + nb * 512 + ns * 128:
                                   ct * N_FULL + nb * 512 + (ns + 1) * 128],
                                wxx_sb[:, ct * C:(ct + 1) * C],
                                start=False, stop=False)

                    def s_c(ns=ns, state=state):
                        yp = state["yp"]
                        oT = oT_cell[0]
                        for et in range(NET):
                            nc.tensor.matmul(
                                yp, oT[:, et * 512 + ns * 128:
                                       et * 512 + (ns + 1) * 128],
                                w1x_sb[:, et * C:(et + 1) * C],
                                start=False, stop=(et == NET - 1))
                        yb = sp.tile([128, C], FP, tag="yb")
                        nc.vector.tensor_add(yb, yp, bcbc_sb)
                        ys = sp.tile([128, C], BF, tag="ys")
                        nc.scalar.activation(ys, yb, AF.Relu)
                        nc.sync.dma_start(
                            y_d[nb * 512 + ns * 128: nb * 512 + (ns + 1) * 128, :],
                            ys)

                    a_steps.append(s_a)
                    b_steps.append(s_b)
                    c_steps.append(s_c)
                return a_steps, b_steps, c_steps

            for _rep in range(reps):
                # ---- merged projections + first query block's s/exp/u pipeline ----
                with (
                    tc.tile_pool(name="psA_fg", bufs=1, space="PSUM") as pfg,
                    tc.tile_pool(name="psA_hv", bufs=1, space="PSUM") as phv,
                ):
                    up0 = pu.tile([128, 2048], FP, tag="u")
                    for ch in range(8):
                        if ch < 7 and _rep == 0:
                            dma_xt(ch + 1)   # prefetch next chunk
                        emit_fg(ch, pfg)
                        emit_hv(4 * ch, phv)
                        emit_hv(4 * ch + 1, phv)
                        emit_su(0, 2 * ch, up0)
                        emit_hv(4 * ch + 2, phv)
                        emit_hv(4 * ch + 3, phv)
                        if ch == 1 and _rep == 0:
                            ident = cpool.tile([128, 128], BF)
                            nc.sync.dma_start(ident, ident_d)
                            w1x_sb = cpool.tile([128, NET * C], BF)
                            nc.sync.dma_start(
                                w1x_sb.rearrange("p (t d) -> p t d", t=NET),
                                w1x_d.rearrange("(t p) d -> p t d", p=128))
                        if ch == 3 and _rep == 0:
                            wxx_sb = cpool.tile([128, NCT * C], BF)
                            nc.sync.dma_start(
                                wxx_sb.rearrange("p (t d) -> p t d", t=NCT),
                                wxx_d.rearrange("(t p) d -> p t d", p=128))
                            bcbc_sb = cpool.tile([128, C], FP)
                            nc.sync.dma_start(bcbc_sb, bcbc_d)
                        emit_su(0, 2 * ch + 1, up0)
                    flush_su(up0)

                # ---- remaining query blocks, y(nb-1) spread into u-loop(nb).
                # Per block: normalize prev-u on DVE first (emit_ob, correct
                # WAR direction vs the recycled up banks), run 2 su chunks,
                # then the prev block's transposes (ob ready by then), then
                # the rest of the su loop with y-steps as queue filler.
                with tc.tile_pool(name="psB_m", bufs=2, space="PSUM") as pm:
                    up_prev = up0
                    for nb in range(1, NNB):
                        obs = emit_ob(up_prev)
                        up = pu.tile([128, 2048], FP, tag="u")
                        for mt2 in range(4):
                            emit_su(nb, mt2, up)
                        oT_cell = [None]
                        emit_oT(obs, oT_cell, pm)
                        a, b, c = make_y_steps(nb - 1, oT_cell, pm)
                        y_steps = [a[0], a[1], b[0], b[1], c[0], c[1],
                                   a[2], a[3], b[2], b[3], c[2], c[3]]
                        for mt2 in range(4, NMT // 2):
                            emit_su(nb, mt2, up)
                            if y_steps:
                                y_steps.pop(0)()
                        flush_su(up)
                        for step in y_steps:
                            step()
                        up_prev = up
                    # last block
                    obs = emit_ob(up_prev)
                    oT_cell = [None]
                    emit_oT(obs, oT_cell, pm)
                    a, b, c = make_y_steps(NNB - 1, oT_cell, pm)
                    for step in [a[0], a[1], b[0], b[1], c[0], c[1],
                                 a[2], a[3], b[2], b[3], c[2], c[3]]:
                        step()

    nc.compile()
    return nc


_PROG = None


def _get_prog():
    global _PROG
    if _PROG is None:
        _PROG = build_program()
    return _PROG


def make_in_maps(x, wf, bf, wg, bg, wh, bh, wo, bo, gamma, wc, bc,
                 bn_scale, bn_bias, bn_mean, bn_var):
    f32 = lambda a: np.ascontiguousarray(np.asarray(a, dtype=np.float32))
    b16 = lambda a: np.ascontiguousarray(np.asarray(a, dtype=np.float64).astype(BF_NP))
    x = f32(x)
    B = x.shape[0]
    xf = x.reshape(B, N_FULL, C)
    gv = float(np.asarray(gamma).ravel()[0])
    sp_ = np.asarray(bn_scale, np.float64) / np.sqrt(np.asarray(bn_var, np.float64) + EPS)
    wcx = np.asarray(wc, np.float64) * sp_[None, :]      # [2C, C], BN-folded
    wcx1, wcx2 = wcx[:C], wcx[C:]
    w1 = gv * (np.asarray(wo, np.float64) @ wcx1)        # [C/2, C]
    wxx = wcx1 + wcx2                                    # [C, C]
    brow = (gv * (np.asarray(bo, np.float64) @ wcx1)
            + (np.asarray(bc, np.float64) - np.asarray(bn_mean, np.float64)) * sp_
            + np.asarray(bn_bias, np.float64))
    whx = np.concatenate([np.asarray(wh, np.float64),
                          np.zeros((C, 2))], axis=1)
    bh_row = np.concatenate([np.asarray(bh, np.float64).ravel(),
                             [1.0, 0.0]])
    common = dict(
        wfg=b16(np.concatenate([np.asarray(wf, np.float32),
                                np.asarray(wg, np.float32)], axis=1)),
        bfg=f32(np.concatenate([np.asarray(bf, np.float32).ravel(),
                                np.asarray(bg, np.float32).ravel()])).reshape(128, 1),
        whx=b16(whx),
        bhbc=f32(np.broadcast_to(bh_row, (128, HW2))),
        w1x=b16(w1),
        wxx=b16(wxx),
        bcbc=f32(np.broadcast_to(brow[None, :], (128, C))),
        identr=np.eye(128, dtype=BF_NP),
    )
    in_maps = []
    for core in range(8):
        b, h = core // 2, core % 2
        own = xf[b, h * N_OWN:(h + 1) * N_OWN]
        oth = xf[b, (1 - h) * N_OWN:(2 - h) * N_OWN]
        xp = np.concatenate([own, oth], axis=0)
        in_maps.append({"xt": np.ascontiguousarray(xp.T.astype(BF_NP)), **common})
    return in_maps, B


def assemble(results, B):
    out = np.empty((B, N_FULL, C), np.float32)
    for core in range(8):
        b, h = core // 2, core % 2
        out[b, h * N_OWN:(h + 1) * N_OWN] = results[core]["y"].astype(np.float32)
    return out.reshape(B, 64, 64, C)


def kernel(**inputs):
    in_maps, B = make_in_maps(**inputs)
    nc = _get_prog()
    res = run_bass_kernel_spmd(nc, in_maps, core_ids=list(range(8)))
    return assemble(res.results, B)
